# revision 1
# baseline (speedup 1.0000x reference)
"""Weighted-MSE loss kernel (nn_LossWithEuler) for 8 Trainium2 NeuronCores.

loss = mean(weight[b] * (inp[d,b] - label[d,b])^2)
  weight[b]  = attr_w[b] * angle_w[b]
  attr_w[b]  = sum_j (attribute[j,b]==1) * (sum(attribute_num)/attribute_num[j])
  angle_w[b] = sum_j (1 - cos(ea[j,b])) = sum_j 2*sin(ea[j,b]/2)^2

Sharding: batch axis B=131072 split across 8 cores (16384 each). Each core's
shard is host-transposed to (16384, 136) so that b sits on SBUF partitions:
partition p holds b in [p*128, (p+1)*128) as 128 contiguous rows of 136 floats.
Per-core partial sums [128,1] are combined on the host.
"""

import sys
import numpy as np

D = 136
B = 131072
N_CORES = 8
BS = B // N_CORES  # 16384 b's per core
P = 128            # SBUF partitions
Q = BS // P        # 128 b's per partition
NCHUNK = 8         # main-loop chunks over the free dim
CB = Q // NCHUNK   # 32 b's per chunk
CF = CB * D        # 4352 free elements per chunk

_program = None


def _build_program():
    try:
        import concourse.bass as bass
    except ImportError:
        sys.path.insert(0, "/opt/trn_rl_repo")
        import concourse.bass as bass
    from concourse import bacc, mybir, tile

    f32 = mybir.dt.float32
    i32 = mybir.dt.int32
    AF = mybir.ActivationFunctionType
    OP = mybir.AluOpType
    AX = mybir.AxisListType

    nc = bacc.Bacc("TRN2", target_bir_lowering=False, debug=False,
                   num_devices=N_CORES)

    # inp and label shards stacked on the host: data[0]=inp.T, data[1]=label.T
    data = nc.dram_tensor("data", (2, BS, D), f32, kind="ExternalInput")
    ea = nc.dram_tensor("ea", (3, BS), f32, kind="ExternalInput")
    attr = nc.dram_tensor("attr", (6, BS), i32, kind="ExternalInput")
    # attribute_num replicated to all 128 partitions on the host; inv_freq is
    # computed redundantly per partition (avoids an on-device broadcast).
    anum = nc.dram_tensor("anum", (P, 6), f32, kind="ExternalInput")
    out = nc.dram_tensor("out", (P, 1), f32, kind="ExternalOutput")

    # (2, BS, D) viewed as [128 partitions, tensor(2), Q*D free]: per chunk a
    # single DMA loads the inp AND label slices (one wait on the consumer).
    data_v = data.ap().rearrange("t (p q) d -> p t (q d)", p=P)

    with tile.TileContext(nc) as tc:
        with tc.tile_pool(name="const", bufs=1) as cpool, \
             tc.tile_pool(name="main", bufs=4) as mpool, \
             tc.tile_pool(name="diffp", bufs=3) as dpool:
            # ---- main-loop data DMAs: chunk 0 and 1 issued first so the
            # critical-path HBM stream starts as early as possible.
            def chunk_dma(c):
                t = mpool.tile([P, 2 * CF], f32, tag="data")
                nc.sync.dma_start(
                    t[:].rearrange("p (t f) -> p t f", t=2),
                    data_v[:, :, c * CF:(c + 1) * CF],
                )
                return t

            dts = {0: chunk_dma(0), 1: chunk_dma(1)}

            # ---- small weight-input DMAs (queued behind chunks 0/1) ----
            a_sb = cpool.tile([P, 6], f32)
            nc.sync.dma_start(a_sb[:], anum.ap())
            attr_i = cpool.tile([P, 6 * Q], i32)
            nc.sync.dma_start(
                attr_i[:].rearrange("p (j q) -> p j q", q=Q),
                attr.ap().rearrange("j (p q) -> p j q", q=Q),
            )
            ea_sb = cpool.tile([P, 3 * Q], f32)
            nc.sync.dma_start(
                ea_sb[:].rearrange("p (j q) -> p j q", q=Q),
                ea.ap().rearrange("j (p q) -> p j q", q=Q),
            )

            # ---- main loop, software-pipelined: DVE stream is
            # sub_0, sub_1, tr_0, sub_2, tr_1, ... so each chunk's ACT square
            # overlaps the next chunk's subtract instead of serializing.
            colsq = cpool.tile([P, Q], f32)
            dfs = {}
            for c in range(NCHUNK):
                if c + 2 < NCHUNK:
                    dts[c + 2] = chunk_dma(c + 2)
                dt_ = dts[c]
                df = dpool.tile([P, CF], f32, tag="diff")
                dfs[c] = df
                # Logical-priority ticks: force the scheduler to place
                # chunk c-1's reduce AFTER chunk c's subtract in the DVE
                # stream, so the ACT square overlaps the next subtract.
                with tc.tile_wait_until(0.004 * c):
                    nc.vector.tensor_sub(df[:], dt_[:, 0:CF], dt_[:, CF:2 * CF])
                if c >= 1:
                    with tc.tile_wait_until(0.004 * c):
                        nc.scalar.activation(
                            dfs[c - 1][:], dfs[c - 1][:], AF.Square)
                    with tc.tile_wait_until(0.004 * c + 0.002):
                        nc.vector.tensor_reduce(
                            colsq[:, (c - 1) * CB:c * CB],
                            dfs[c - 1][:].rearrange("p (b d) -> p b d", d=D),
                            axis=AX.X, op=OP.add,
                        )
            c = NCHUNK - 1
            nc.scalar.activation(dfs[c][:], dfs[c][:], AF.Square)
            nc.vector.tensor_reduce(
                colsq[:, c * CB:(c + 1) * CB],
                dfs[c][:].rearrange("p (b d) -> p b d", d=D),
                axis=AX.X, op=OP.add,
            )

            # ---- weight computation (inputs landed long ago; these small ops
            # run at the tail of each engine's stream).
            # inverse-frequency: ivb[p,j] = sum(anum)/anum[j]
            tot = cpool.tile([P, 1], f32)
            nc.vector.tensor_reduce(tot[:], a_sb[:], axis=AX.X, op=OP.add)
            rec = cpool.tile([P, 6], f32)
            nc.vector.reciprocal(rec[:], a_sb[:])
            ivb = cpool.tile([P, 6], f32)
            nc.vector.tensor_scalar_mul(ivb[:], rec[:], tot[:, 0:1])
            # attr_w[p,q] = sum_j attr[j, p*128+q] * iv[j]
            attr_f = cpool.tile([P, 6 * Q], f32)
            nc.vector.tensor_copy(attr_f[:], attr_i[:])
            aw0 = cpool.tile([P, Q], f32)
            aw1 = cpool.tile([P, Q], f32)
            nc.vector.tensor_scalar_mul(aw0[:], attr_f[:, 0:Q], ivb[:, 0:1])
            cur, nxt = aw0, aw1
            for j in range(1, 6):
                nc.vector.scalar_tensor_tensor(
                    nxt[:], attr_f[:, j * Q:(j + 1) * Q], ivb[:, j:j + 1],
                    cur[:], op0=OP.mult, op1=OP.add,
                )
                cur, nxt = nxt, cur
            aw = cur
            # angle_w[p,q] = 2 * sum_j sin(ea[j, p*128+q]/2)^2
            sinh_sb = cpool.tile([P, 3 * Q], f32)
            nc.scalar.activation(sinh_sb[:], ea_sb[:], AF.Sin, bias=0.0, scale=0.5)
            ssq = cpool.tile([P, 3 * Q], f32)
            nc.vector.tensor_mul(ssq[:], sinh_sb[:], sinh_sb[:])
            angle = cpool.tile([P, Q], f32)
            nc.vector.tensor_reduce(
                angle[:], ssq[:].rearrange("p (j q) -> p q j", q=Q),
                axis=AX.X, op=OP.add,
            )
            # weight[p,q] = (2*angle) * attr_w
            w_sb = cpool.tile([P, Q], f32)
            nc.vector.scalar_tensor_tensor(
                w_sb[:], angle[:], 2.0, aw[:], op0=OP.mult, op1=OP.mult,
            )

            # ---- partial[p] = sum_q colsq[p,q] * weight[p,q] ----
            scr = cpool.tile([P, Q], f32)
            part = cpool.tile([P, 1], f32)
            nc.vector.tensor_mul(scr[:], colsq[:], w_sb[:])
            nc.vector.tensor_reduce(part[:], scr[:], axis=AX.X, op=OP.add)
            nc.sync.dma_start(out.ap(), part[:])

    nc.compile()
    return nc


def _get_program():
    global _program
    if _program is None:
        _program = _build_program()
    return _program


def _make_in_maps(inp, label, ea, attribute, attribute_num):
    inp = np.asarray(inp, dtype=np.float32)
    label = np.asarray(label, dtype=np.float32)
    ea = np.asarray(ea, dtype=np.float32)
    attribute = np.asarray(attribute, dtype=np.int32)
    anum = np.tile(np.asarray(attribute_num, dtype=np.float32).reshape(1, 6),
                   (P, 1))
    in_maps = []
    for c in range(N_CORES):
        s = slice(c * BS, (c + 1) * BS)
        dat = np.empty((2, BS, D), dtype=np.float32)
        dat[0] = inp[:, s].T
        dat[1] = label[:, s].T
        in_maps.append({
            "data": dat,
            "ea": np.ascontiguousarray(ea[:, s]),
            "attr": np.ascontiguousarray(attribute[:, s]),
            "anum": anum,
        })
    return in_maps


def run(inputs, trace=False, trace_cores=None):
    """Run on hardware; returns (result_scalar, BassKernelResults)."""
    try:
        from concourse.bass_utils import run_bass_kernel_spmd
    except ImportError:
        sys.path.insert(0, "/opt/trn_rl_repo")
        from concourse.bass_utils import run_bass_kernel_spmd
    nc = _get_program()
    in_maps = _make_in_maps(**inputs)
    kwargs = {}
    if trace:
        kwargs["trace"] = True
        if trace_cores is not None:
            kwargs["trace_cores"] = trace_cores
    res = run_bass_kernel_spmd(nc, in_maps, core_ids=list(range(N_CORES)), **kwargs)
    total = 0.0
    for r in res.results:
        total += r["out"].astype(np.float64).sum()
    value = np.asarray(total / (D * B), dtype=np.float32)
    return value, res


def kernel(**inputs):
    value, _ = run(inputs)
    return value



# revision 5
# speedup vs baseline: 1.2259x; 1.2259x over previous
"""Weighted-MSE loss kernel (nn_LossWithEuler) for 8 Trainium2 NeuronCores.

loss = mean(weight[b] * (inp[d,b] - label[d,b])^2)
  weight[b]  = attr_w[b] * angle_w[b]
  attr_w[b]  = sum_j (attribute[j,b]==1) * (sum(attribute_num)/attribute_num[j])
  angle_w[b] = sum_j (1 - cos(ea[j,b])) = sum_j 2*sin(ea[j,b]/2)^2

Sharding: batch axis B=131072 split across 8 cores (16384 each). Each core's
shard is host-packed to partition-major layout: partition p holds b-locals
[p*128, (p+1)*128), stored chunk-interleaved so every chunk's inp+label slice
is one contiguous run per partition (one large DMA descriptor each).

Per-core dataflow: chunked DVE subtract -> ACT square (in place) -> DVE
per-sample reduce; chunk sizes taper (16,16,16,16,16,16,8,8,4,4,4,4 samples
per partition) so the final chunk's compute tail after the last DMA is short.
The DVE instruction order is pinned (sub(c+1) before reduce(c)) so the ACT
square of chunk c overlaps the subtract of chunk c+1. Attribute/Euler weight
math runs on the otherwise-idle GpSimd engine. Per-partition partials are
reduced across partitions on the TensorEngine (ones-vector matmul) so each
core writes a single f32 (one DMA descriptor) instead of a 128-descriptor
spray.
"""

import sys
import numpy as np

D = 136
B = 131072
N_CORES = 8
BS = B // N_CORES  # 16384 b's per core
P = 128            # SBUF partitions
Q = BS // P        # 128 b's per partition
# graduated chunk sizes (in b's per partition); sum must equal Q
CHUNK_B = [16, 16, 16, 16, 16, 16, 8, 8, 4, 4, 4, 4]
assert sum(CHUNK_B) == Q
NCHUNK = len(CHUNK_B)
TOT_F = 2 * Q * D  # f32 elements per partition in the packed data tensor

_program = None


def _build_program():
    try:
        import concourse.bass as bass  # noqa: F401
    except ImportError:
        sys.path.insert(0, "/opt/trn_rl_repo")
        import concourse.bass as bass  # noqa: F401
    from concourse import bacc, mybir, tile
    from concourse.tile import add_dep_helper

    f32 = mybir.dt.float32
    i32 = mybir.dt.int32
    AF = mybir.ActivationFunctionType
    OP = mybir.AluOpType
    AX = mybir.AxisListType

    nc = bacc.Bacc("TRN2", target_bir_lowering=False, debug=False,
                   num_devices=N_CORES)

    data = nc.dram_tensor("data", (P, TOT_F), f32, kind="ExternalInput")
    ea = nc.dram_tensor("ea", (3, BS), f32, kind="ExternalInput")
    attr = nc.dram_tensor("attr", (6, BS), i32, kind="ExternalInput")
    # attribute_num replicated to all 128 partitions on the host; inv_freq is
    # computed redundantly per partition (avoids an on-device broadcast).
    anum = nc.dram_tensor("anum", (P, 6), f32, kind="ExternalInput")
    out = nc.dram_tensor("out", (1, 1), f32, kind="ExternalOutput")

    with tile.TileContext(nc) as tc:
        with tc.tile_pool(name="const", bufs=1) as cpool, \
             tc.tile_pool(name="data16", bufs=6) as p16, \
             tc.tile_pool(name="data8", bufs=2) as p8, \
             tc.tile_pool(name="data4", bufs=4) as p4, \
             tc.tile_pool(name="psum", bufs=1, space="PSUM") as ppool:
            # ---- small weight-input DMAs first (HWDGE FIFO: they complete
            # before the chunk stream, unblocking the GpSimd weight math).
            a_sb = cpool.tile([P, 6], f32)
            nc.sync.dma_start(a_sb[:], anum.ap())
            attr_i = cpool.tile([P, 6 * Q], i32)
            nc.sync.dma_start(
                attr_i[:].rearrange("p (j q) -> p j q", q=Q),
                attr.ap().rearrange("j (p q) -> p j q", q=Q),
            )
            ea_sb = cpool.tile([P, 3 * Q], f32)
            nc.sync.dma_start(
                ea_sb[:].rearrange("p (j q) -> p j q", q=Q),
                ea.ap().rearrange("j (p q) -> p j q", q=Q),
            )

            # ---- all chunk DMAs issued upfront; every chunk has its own
            # resident tile so the HWDGE stream never waits on compute.
            pools = {16: p16, 8: p8, 4: p4}
            dts = []
            dmas = []
            off = 0
            for cb in CHUNK_B:
                f = cb * D
                t = pools[cb].tile([P, 2 * f], f32, tag=f"d{cb}")
                dmas.append(nc.sync.dma_start(
                    t[:], data.ap()[:, off:off + 2 * f]))
                dts.append(t)
                off += 2 * f
            for i in range(len(dmas) - 1):
                add_dep_helper(dmas[i + 1].ins, dmas[i].ins, sync=False,
                               reason="chunk DMA issue order")

            # ---- weight computation on GpSimd (idle engine) + ACT sin.
            # inverse-frequency: ivb[p,j] = sum(anum)/anum[j]
            tot = cpool.tile([P, 1], f32)
            nc.vector.tensor_reduce(tot[:], a_sb[:], axis=AX.X, op=OP.add)
            rec = cpool.tile([P, 6], f32)
            nc.vector.reciprocal(rec[:], a_sb[:])
            ivb = cpool.tile([P, 6], f32)
            nc.vector.tensor_scalar_mul(ivb[:], rec[:], tot[:, 0:1])
            # attr_w[p,q] = sum_j attr[j, p*128+q] * iv[j]
            attr_f = cpool.tile([P, 6 * Q], f32)
            nc.vector.tensor_copy(attr_f[:], attr_i[:])
            aw0 = cpool.tile([P, Q], f32)
            aw1 = cpool.tile([P, Q], f32)
            nc.vector.tensor_scalar_mul(aw0[:], attr_f[:, 0:Q], ivb[:, 0:1])
            cur, nxt = aw0, aw1
            for j in range(1, 6):
                nc.vector.scalar_tensor_tensor(
                    nxt[:], attr_f[:, j * Q:(j + 1) * Q], ivb[:, j:j + 1],
                    cur[:], op0=OP.mult, op1=OP.add,
                )
                cur, nxt = nxt, cur
            aw = cur
            # angle_w[p,q] = 2 * sum_j sin(ea[j, p*128+q]/2)^2
            sinh_sb = cpool.tile([P, 3 * Q], f32)
            nc.scalar.activation(sinh_sb[:], ea_sb[:], AF.Sin, bias=0.0,
                                 scale=0.5)
            ssq = cpool.tile([P, 3 * Q], f32)
            nc.vector.tensor_mul(ssq[:], sinh_sb[:], sinh_sb[:])
            angle = cpool.tile([P, Q], f32)
            nc.vector.tensor_reduce(
                angle[:], ssq[:].rearrange("p (j q) -> p q j", q=Q),
                axis=AX.X, op=OP.add,
            )
            # weight[p,q] = (2*angle) * attr_w
            w_sb = cpool.tile([P, Q], f32)
            nc.vector.scalar_tensor_tensor(
                w_sb[:], angle[:], 2.0, aw[:], op0=OP.mult, op1=OP.mult,
            )
            # ones vector for the final cross-partition matmul reduce
            ones = cpool.tile([P, 1], f32)
            nc.gpsimd.memset(ones[:], 1.0)

            # ---- main loop: diff over the label half in place, square in
            # place on ACT, per-sample d-reduce into colsq columns.
            colsq = cpool.tile([P, Q], f32)
            subs = []
            reds = []
            q0 = 0
            for c, cb in enumerate(CHUNK_B):
                f = cb * D
                dt_ = dts[c]
                subs.append(nc.vector.tensor_sub(
                    dt_[:, f:2 * f], dt_[:, 0:f], dt_[:, f:2 * f]))
                nc.scalar.activation(dt_[:, f:2 * f], dt_[:, f:2 * f],
                                     AF.Square)
                reds.append(nc.vector.tensor_reduce(
                    colsq[:, q0:q0 + cb],
                    dt_[:, f:2 * f].rearrange("p (b d) -> p b d", d=D),
                    axis=AX.X, op=OP.add,
                ))
                q0 += cb
            # Pin the DVE stream to sub0, sub1, red0, sub2, red1, ... so each
            # chunk's ACT square overlaps the next chunk's subtract (the
            # scheduler's DMA cost model would otherwise serialize them).
            for c in range(NCHUNK - 1):
                add_dep_helper(reds[c].ins, subs[c + 1].ins, sync=False,
                               reason="pipeline: sub(c+1) before red(c)")
                if c + 2 < NCHUNK:
                    add_dep_helper(subs[c + 2].ins, reds[c].ins, sync=False,
                                   reason="pipeline: red(c) before sub(c+2)")

            # ---- partial[p] = sum_q colsq[p,q] * weight[p,q], then reduce
            # across partitions on the TensorEngine and write one f32.
            scr = cpool.tile([P, Q], f32)
            part = cpool.tile([P, 1], f32)
            nc.vector.tensor_mul(scr[:], colsq[:], w_sb[:])
            nc.vector.tensor_reduce(part[:], scr[:], axis=AX.X, op=OP.add)
            ps = ppool.tile([1, 1], f32)
            nc.tensor.matmul(ps[:], ones[:], part[:], start=True, stop=True)
            res = cpool.tile([1, 1], f32)
            nc.vector.tensor_copy(res[:], ps[:])
            nc.sync.dma_start(out.ap(), res[:])

    nc.compile()
    return nc


def _get_program():
    global _program
    if _program is None:
        _program = _build_program()
    return _program


def _make_in_maps(inp, label, ea, attribute, attribute_num):
    inp = np.asarray(inp, dtype=np.float32)
    label = np.asarray(label, dtype=np.float32)
    ea = np.asarray(ea, dtype=np.float32)
    attribute = np.asarray(attribute, dtype=np.int32)
    anum = np.tile(np.asarray(attribute_num, dtype=np.float32).reshape(1, 6),
                   (P, 1))
    in_maps = []
    for c in range(N_CORES):
        s = slice(c * BS, (c + 1) * BS)
        it = np.ascontiguousarray(inp[:, s].T).reshape(P, Q, D)
        lt = np.ascontiguousarray(label[:, s].T).reshape(P, Q, D)
        dat = np.empty((P, TOT_F), dtype=np.float32)
        off = 0
        q0 = 0
        for cb in CHUNK_B:
            f = cb * D
            dat[:, off:off + f] = it[:, q0:q0 + cb].reshape(P, f)
            dat[:, off + f:off + 2 * f] = lt[:, q0:q0 + cb].reshape(P, f)
            off += 2 * f
            q0 += cb
        in_maps.append({
            "data": dat,
            "ea": np.ascontiguousarray(ea[:, s]),
            "attr": np.ascontiguousarray(attribute[:, s]),
            "anum": anum,
        })
    return in_maps


def run(inputs, trace=False, trace_cores=None):
    """Run on hardware; returns (result_scalar, BassKernelResults)."""
    try:
        from concourse.bass_utils import run_bass_kernel_spmd
    except ImportError:
        sys.path.insert(0, "/opt/trn_rl_repo")
        from concourse.bass_utils import run_bass_kernel_spmd
    nc = _get_program()
    in_maps = _make_in_maps(**inputs)
    kwargs = {}
    if trace:
        kwargs["trace"] = True
        if trace_cores is not None:
            kwargs["trace_cores"] = trace_cores
    res = run_bass_kernel_spmd(nc, in_maps, core_ids=list(range(N_CORES)), **kwargs)
    total = 0.0
    for r in res.results:
        total += float(r["out"].astype(np.float64).sum())
    value = np.asarray(total / (D * B), dtype=np.float32)
    return value, res


def kernel(**inputs):
    value, _ = run(inputs)
    return value


# revision 12
# speedup vs baseline: 1.2843x; 1.0477x over previous
"""Weighted-MSE loss kernel (nn_LossWithEuler) for 8 Trainium2 NeuronCores.

loss = mean(weight[b] * (inp[d,b] - label[d,b])^2)
  weight[b]  = attr_w[b] * angle_w[b]
  attr_w[b]  = sum_j (attribute[j,b]==1) * (sum(attribute_num)/attribute_num[j])
  angle_w[b] = sum_j (1 - cos(ea[j,b])) = sum_j 2*sin(ea[j,b]/2)^2

Sharding: batch axis B=131072 split across 8 cores (16384 each). Each core's
shard is host-packed to partition-major layout: partition p holds b-locals
[p*128, (p+1)*128), stored chunk-interleaved so every chunk's inp+label slice
is one contiguous run per partition (one large DMA descriptor each). The
small inputs (ea/attribute/attribute_num) are packed into a single "aux"
tensor (attribute bit-cast into the f32 stream) so they cost one DMA.

Per-core dataflow: chunked DVE subtract -> ACT square (bf16 out) -> DVE
per-sample reduce; chunk sizes taper (16x6,8x3,4,2,2 samples per partition)
so the final chunk's compute tail after the last DMA is short. The DVE
instruction order is pinned (sub(c+1) before reduce(c)) so the ACT square of
chunk c overlaps the subtract of chunk c+1. Per-partition partials are
reduced across partitions on the TensorEngine (ones-vector matmul) so each
core writes a single f32 (one DMA descriptor).
"""

import sys
import numpy as np

D = 136
B = 131072
N_CORES = 8
BS = B // N_CORES  # 16384 b's per core
P = 128            # SBUF partitions
Q = BS // P        # 128 b's per partition
# graduated chunk sizes (in b's per partition); sum must equal Q
CHUNK_B = [16, 16, 16, 16, 16, 16, 8, 8, 8, 4, 2, 2]
assert sum(CHUNK_B) == Q
NCHUNK = len(CHUNK_B)
TOT_F = 2 * Q * D   # f32 elements per partition in the packed data tensor
AUX_F = 3 * Q + 6 * Q + 6  # ea + attr(bitcast) + anum, f32 words / partition

_program = None


def _build_program():
    try:
        import concourse.bass as bass  # noqa: F401
    except ImportError:
        sys.path.insert(0, "/opt/trn_rl_repo")
        import concourse.bass as bass  # noqa: F401
    from concourse import bacc, mybir, tile
    from concourse.tile import add_dep_helper

    f32 = mybir.dt.float32
    i32 = mybir.dt.int32
    bf16 = mybir.dt.bfloat16
    AF = mybir.ActivationFunctionType
    OP = mybir.AluOpType
    AX = mybir.AxisListType

    nc = bacc.Bacc("TRN2", target_bir_lowering=False, debug=False,
                   num_devices=N_CORES)

    data = nc.dram_tensor("data", (P, TOT_F), f32, kind="ExternalInput")
    aux = nc.dram_tensor("aux", (P, AUX_F), f32, kind="ExternalInput")
    out = nc.dram_tensor("out", (1, 1), f32, kind="ExternalOutput")

    with tile.TileContext(nc) as tc:
        with tc.tile_pool(name="const", bufs=1) as cpool, \
             tc.tile_pool(name="data16", bufs=6) as p16, \
             tc.tile_pool(name="data8", bufs=3) as p8, \
             tc.tile_pool(name="data4", bufs=1) as p4, \
             tc.tile_pool(name="data2", bufs=2) as p2, \
             tc.tile_pool(name="sq", bufs=3) as qpool, \
             tc.tile_pool(name="psum", bufs=1, space="PSUM") as ppool:
            # ---- DMA queue (HWDGE FIFO): chunk0, chunk1, aux, chunk2.. so
            # the bulk stream starts immediately and weight inputs land early.
            pools = {16: p16, 8: p8, 4: p4, 2: p2}
            dts = []
            dmas = []
            off = 0
            for c, cb in enumerate(CHUNK_B):
                f = cb * D
                t = pools[cb].tile([P, 2 * f], f32, tag=f"d{cb}")
                dts.append(t)
                dmas.append(nc.sync.dma_start(
                    t[:], data.ap()[:, off:off + 2 * f]))
                off += 2 * f
                if c == 1:
                    aux_sb = cpool.tile([P, AUX_F], f32)
                    dmas.append(nc.sync.dma_start(aux_sb[:], aux.ap()))
            for i in range(len(dmas) - 1):
                add_dep_helper(dmas[i + 1].ins, dmas[i].ins, sync=False,
                               reason="DMA issue order")
            ea_sb = aux_sb[:, 0:3 * Q]
            attr_f32v = aux_sb[:, 3 * Q:9 * Q]
            a_sb = aux_sb[:, 9 * Q:9 * Q + 6]

            # ---- weight computation (DVE + one ACT sin); runs in the DVE
            # idle window while the first data chunks stream in.
            # inverse-frequency: ivb[p,j] = sum(anum)/anum[j]
            tot = cpool.tile([P, 1], f32)
            nc.vector.tensor_reduce(tot[:], a_sb, axis=AX.X, op=OP.add)
            rec = cpool.tile([P, 6], f32)
            nc.vector.reciprocal(rec[:], a_sb)
            ivb = cpool.tile([P, 6], f32)
            nc.vector.tensor_scalar_mul(ivb[:], rec[:], tot[:, 0:1])
            # attr_w[p,q] = sum_j attr[j, p*128+q] * iv[j]
            # (attribute is host-converted to f32 0.0/1.0 inside aux)
            aw0 = cpool.tile([P, Q], f32)
            aw1 = cpool.tile([P, Q], f32)
            nc.vector.tensor_scalar_mul(aw0[:], attr_f32v[:, 0:Q],
                                        ivb[:, 0:1])
            cur, nxt = aw0, aw1
            for j in range(1, 6):
                nc.vector.scalar_tensor_tensor(
                    nxt[:], attr_f32v[:, j * Q:(j + 1) * Q], ivb[:, j:j + 1],
                    cur[:], op0=OP.mult, op1=OP.add,
                )
                cur, nxt = nxt, cur
            aw = cur
            # angle_w[p,q] = 2 * sum_j sin(ea[j, p*128+q]/2)^2
            sinh_sb = cpool.tile([P, 3 * Q], f32)
            nc.scalar.activation(sinh_sb[:], ea_sb, AF.Sin, bias=0.0,
                                 scale=0.5)
            ssq = cpool.tile([P, 3 * Q], f32)
            nc.vector.tensor_mul(ssq[:], sinh_sb[:], sinh_sb[:])
            angle = cpool.tile([P, Q], f32)
            nc.vector.tensor_reduce(
                angle[:], ssq[:].rearrange("p (j q) -> p q j", q=Q),
                axis=AX.X, op=OP.add,
            )
            # weight[p,q] = (2*angle) * attr_w
            w_sb = cpool.tile([P, Q], f32)
            nc.vector.scalar_tensor_tensor(
                w_sb[:], angle[:], 2.0, aw[:], op0=OP.mult, op1=OP.mult,
            )
            # ones vector for the final cross-partition matmul reduce
            ones = cpool.tile([P, 1], f32)
            nc.gpsimd.memset(ones[:], 1.0)

            # ---- main loop: diff over the label half in place, bf16 square
            # on ACT, per-sample d-reduce (16-bit DVE path) into colsq.
            colsq = cpool.tile([P, Q], f32)
            subs = []
            reds = []
            q0 = 0
            for c, cb in enumerate(CHUNK_B):
                f = cb * D
                dt_ = dts[c]
                subs.append(nc.vector.tensor_sub(
                    dt_[:, f:2 * f], dt_[:, 0:f], dt_[:, f:2 * f]))
                sq = qpool.tile([P, f], bf16, tag="sq")
                nc.scalar.activation(sq[:], dt_[:, f:2 * f], AF.Square)
                reds.append(nc.vector.tensor_reduce(
                    colsq[:, q0:q0 + cb],
                    sq[:].rearrange("p (b d) -> p b d", d=D),
                    axis=AX.X, op=OP.add,
                ))
                q0 += cb
            # Pin the DVE stream to sub0, sub1, red0, sub2, red1, ... so each
            # chunk's ACT square overlaps the next chunk's subtract (the
            # scheduler's DMA cost model would otherwise serialize them).
            for c in range(NCHUNK - 1):
                add_dep_helper(reds[c].ins, subs[c + 1].ins, sync=False,
                               reason="pipeline: sub(c+1) before red(c)")
                if c + 2 < NCHUNK:
                    add_dep_helper(subs[c + 2].ins, reds[c].ins, sync=False,
                                   reason="pipeline: red(c) before sub(c+2)")

            # ---- partial[p] = sum_q colsq[p,q] * weight[p,q] (one fused
            # DVE op), then reduce across partitions on the TensorEngine and
            # write a single f32.
            scr = cpool.tile([P, Q], f32)
            part = cpool.tile([P, 1], f32)
            nc.vector.tensor_mul(scr[:], colsq[:], w_sb[:])
            nc.vector.tensor_reduce(part[:], scr[:], axis=AX.X, op=OP.add)
            ps = ppool.tile([1, 1], f32)
            nc.tensor.matmul(ps[:], ones[:], part[:], start=True, stop=True)
            res = cpool.tile([1, 1], f32)
            nc.vector.tensor_copy(res[:], ps[:])
            nc.sync.dma_start(out.ap(), res[:])

    nc.compile()
    return nc


def _get_program():
    global _program
    if _program is None:
        _program = _build_program()
    return _program


def _make_in_maps(inp, label, ea, attribute, attribute_num):
    inp = np.asarray(inp, dtype=np.float32)
    label = np.asarray(label, dtype=np.float32)
    ea = np.asarray(ea, dtype=np.float32)
    attribute = np.asarray(attribute, dtype=np.int32)
    anum_row = np.asarray(attribute_num, dtype=np.float32).reshape(6)
    in_maps = []
    for c in range(N_CORES):
        s = slice(c * BS, (c + 1) * BS)
        it = np.ascontiguousarray(inp[:, s].T).reshape(P, Q, D)
        lt = np.ascontiguousarray(label[:, s].T).reshape(P, Q, D)
        dat = np.empty((P, TOT_F), dtype=np.float32)
        off = 0
        q0 = 0
        for cb in CHUNK_B:
            f = cb * D
            dat[:, off:off + f] = it[:, q0:q0 + cb].reshape(P, f)
            dat[:, off + f:off + 2 * f] = lt[:, q0:q0 + cb].reshape(P, f)
            off += 2 * f
            q0 += cb
        aux = np.empty((P, AUX_F), dtype=np.float32)
        aux[:, 0:3 * Q] = (
            ea[:, s].reshape(3, P, Q).transpose(1, 0, 2).reshape(P, 3 * Q))
        aux[:, 3 * Q:9 * Q] = (
            attribute[:, s].reshape(6, P, Q).transpose(1, 0, 2)
            .reshape(P, 6 * Q).astype(np.float32))
        aux[:, 9 * Q:9 * Q + 6] = anum_row
        in_maps.append({"data": dat, "aux": aux})
    return in_maps


def run(inputs, trace=False, trace_cores=None):
    """Run on hardware; returns (result_scalar, BassKernelResults)."""
    try:
        from concourse.bass_utils import run_bass_kernel_spmd
    except ImportError:
        sys.path.insert(0, "/opt/trn_rl_repo")
        from concourse.bass_utils import run_bass_kernel_spmd
    nc = _get_program()
    in_maps = _make_in_maps(**inputs)
    kwargs = {}
    if trace:
        kwargs["trace"] = True
        if trace_cores is not None:
            kwargs["trace_cores"] = trace_cores
    res = run_bass_kernel_spmd(nc, in_maps, core_ids=list(range(N_CORES)), **kwargs)
    total = 0.0
    for r in res.results:
        total += float(r["out"].astype(np.float64).sum())
    value = np.asarray(total / (D * B), dtype=np.float32)
    return value, res


def kernel(**inputs):
    value, _ = run(inputs)
    return value
